# revision 1
# baseline (speedup 1.0000x reference)
"""Trainium2 Bass kernel for Enformer-style relative-position attention.

Problem: nn_Attention_79087527788690
  x [1, 2048, 1536] -> out [1, 2048, 1536]
  8 heads, dk=64, dv=192, rel-pos features=192, n=2048.

Sharding: one head per NeuronCore (8 cores). Each core computes its head's
q/k/v projections, content + relative-position logits (exp carried through
the DRAM shear), softmax weights and per-head attention output oh
[2048, 192]. Transposed oh row-tiles are exchanged via two AllToAll
collectives (tiles 0-7, then 8-15; core c owns tiles {c, 8+c}); each core
then multiplies its two owned row-tiles against the full (dv-swizzled) Wo
for its final [256, 1536] output rows - no [2048, 1536] partial sums ever
materialize.

The relative_shift is realized with a DRAM round trip: for each q-tile the
rel-logit window G[p, u] (u in a 2175-wide span starting at s0 = 1920 - i0)
is written contiguously to DRAM ([128, 2176] pitch) and read back with a
skewed access pattern (row stride 2175, offset 127), which gives
shifted[p, j] = G[p, 127 - p + j] without cross-partition traffic.
"""

import math
import os
import sys
from contextlib import ExitStack

sys.path.insert(0, "/opt/trn_rl_repo")

import numpy as np

N = 2048
DIM = 1536
HEADS = 8
DK = 64
DV = 192
F = 192  # rel pos features
SPAN = 2 * N - 1  # 4095
NCORES = 8
CHUNK = N // NCORES  # 256
SCALE = DK ** -0.5

K_ETB = os.environ.get("K_ETB", "dve")
K_TRAIL = int(os.environ.get("K_TRAIL", "1"))
K_LEAD = int(os.environ.get("K_LEAD", "4"))
K_EMUL = os.environ.get("K_EMUL", "dve")  # engine for the E multiply
K_ORDER = os.environ.get("K_ORDER", "pv_first")
K_OHT = os.environ.get("K_OHT", "dve")
K_ESPLIT = int(os.environ.get("K_ESPLIT", "1"))
K_RSUM = os.environ.get("K_RSUM", "vcol")
K_EB = int(os.environ.get("K_EB", "4"))
K_GB = int(os.environ.get("K_GB", "5"))
K_RB = int(os.environ.get("K_RB", "5"))
K_DRAIN = os.environ.get("K_DRAIN", "0") == "1"
K_RELKSPLIT = os.environ.get("K_RELKSPLIT", "0") == "1"
K_EMIT0 = os.environ.get("K_EMIT0", "relk_first")
K_DMAORD = os.environ.get("K_DMAORD", "pos_first")
K_PART = os.environ.get("K_PART", "dve")
K_OP0 = os.environ.get("K_OP0", "drain")
K_FB = int(os.environ.get("K_FB", "2"))
K_WOFIT = int(os.environ.get("K_WOFIT", "3"))
K_PPB = int(os.environ.get("K_PPB", "1"))  # outproj psum ring depth
K_PCB = int(os.environ.get("K_PCB", "2"))  # content psum ring depth
K_FINE = os.environ.get("K_FINE", "act")
K_OP0RING = os.environ.get("K_OP0RING", "pp")
K_ILV = os.environ.get("K_ILV", "0") == "1"
K_POB = int(os.environ.get("K_POB", "1"))
K_OPALT = os.environ.get("K_OPALT", "1") == "1"
K_WARM = int(os.environ.get("K_WARM", "0"))
K_GORD = os.environ.get("K_GORD", "0") == "1"
K_POSHI = os.environ.get("K_POSHI", "0") == "1"
K_TRDMA = int(os.environ.get("K_TRDMA", "0"))  # jq groups via dma transpose
K_GPOS = os.environ.get("K_GPOS", "late")
K_GILV = os.environ.get("K_GILV", "0") == "1"
K_COMM0 = int(os.environ.get("K_COMM0", "9"))  # G emission position in iteration
K_TRB = int(os.environ.get("K_TRB", "2"))  # transpose psum ring depth

IT = 128          # q rows per tile
NIT = N // IT     # 16
JC = 512          # j chunk for logits
NJC = N // JC     # 4
GW = N + IT - 1   # 2175, G window per i-tile
GPITCH = 2176     # padded pitch of the DRAM G buffer
NAG = 2           # all-to-all groups (tiles 0-7, 8-15)


def _positions() -> np.ndarray:
    """get_positional_embed(2048, 192) in numpy (f64 -> f32). [4095, 192]"""
    d = np.arange(-N + 1, N).astype(np.float64)
    nb = F // 6
    absd = np.abs(d)[:, None]
    max_range = math.log(N) / math.log(2.0)
    half_life = 2.0 ** np.linspace(3.0, max_range, nb)
    feat_exp = np.exp(-math.log(2.0) / half_life[None, :] * absd)
    cw = 2.0 ** np.arange(1, nb + 1) - 1.0
    feat_cm = (cw[None, :] > absd).astype(np.float64)
    stddev = N / (2 * nb)
    start_mean = N / nb
    mean = np.linspace(start_mean, N, nb)[None, :]
    conc = (mean / stddev) ** 2
    rate = mean / stddev ** 2
    with np.errstate(divide="ignore", invalid="ignore"):
        log_unnorm = (conc - 1.0) * np.log(absd) - rate * absd
    log_unnorm = np.where(absd == 0, -np.inf, log_unnorm)
    lg = np.vectorize(math.lgamma)(conc)
    log_norm = lg - conc * np.log(rate)
    probs = np.exp(log_unnorm - log_norm) + 1e-8
    feat_gamma = probs / np.amax(probs, axis=-1, keepdims=True)
    emb = np.concatenate([feat_exp, feat_cm, feat_gamma], axis=-1)
    out = np.concatenate([emb, np.sign(d)[:, None] * emb], axis=-1)
    return out.astype(np.float32)


def build_nc(num_cores: int = NCORES, collective: bool = True):
    """Build + compile the per-core Bass graph (SPMD, identical on all cores)."""
    import concourse.bass as bass
    import concourse.mybir as mybir
    import concourse.tile as tile
    from concourse import bacc
    from concourse.masks import make_identity

    f32 = mybir.dt.float32
    f16 = mybir.dt.float16
    bf16 = mybir.dt.bfloat16

    nc = bacc.Bacc(
        "TRN2", target_bir_lowering=False, debug=False, num_devices=num_cores
    )

    # --- external I/O (per-core shards supplied via in_maps) ---
    xT_e = nc.dram_tensor("xT", [DIM, N], f16, kind="ExternalInput")
    # wqk: [p, c, 0:64] = wq[128c+p] * SCALE ; [p, c, 64:128] = wk[128c+p]
    wqk_e = nc.dram_tensor("wqk", [128, 12, 2 * DK], f16, kind="ExternalInput")
    # wvp: [p, c, :] = wv[128c+p]
    wvp_e = nc.dram_tensor("wvp", [128, 12, DV], f16, kind="ExternalInput")
    # wrel2: [p, 0:64] = wrel[p] ; [p<64, 64:128] = wrel[128+p]
    wrel2_e = nc.dram_tensor("wrel2", [128, 2 * DK], f16, kind="ExternalInput")
    posT_e = nc.dram_tensor("posT", [F, SPAN], f16, kind="ExternalInput")
    rb2_e = nc.dram_tensor("rb2", [DK, 2], f32, kind="ExternalInput")
    # wof: full out-projection weight, dv-swizzled: [p, kc, :] = woP[128kc + p]
    wof_e = nc.dram_tensor("wof", [128, 12, DIM], f16, kind="ExternalInput")
    bo_e = nc.dram_tensor("bo", [1, DIM], f16, kind="ExternalInput")
    out_e = nc.dram_tensor("out", [CHUNK, DIM], f32, kind="ExternalOutput")

    # --- internal DRAM ---
    # a2a slot o of group ag carries this core's ohT for tile 8*ag + o
    a2a_ins = [nc.dram_tensor(f"a2a_in{g}", [NCORES, DV, IT], f16) for g in range(NAG)]
    a2a_outs = [nc.dram_tensor(f"a2a_out{g}", [NCORES, DV, IT], f16) for g in range(NAG)]
    NGD = 5
    gds = [nc.dram_tensor(f"gd{i}", [IT, GPITCH], bf16) for i in range(NGD)]

    with tile.TileContext(nc) as tc, ExitStack() as ctx:
        const = ctx.enter_context(tc.tile_pool(name="const", bufs=1))
        work = ctx.enter_context(tc.tile_pool(name="work", bufs=2))
        psum = ctx.enter_context(tc.tile_pool(name="psum", bufs=2, space="PSUM"))

        # ---- constants / weights into SBUF ----
        ident_b = const.tile([128, 128], bf16, tag="idb")
        make_identity(nc, ident_b[:])
        ident_h = const.tile([128, 128], f16, tag="idh")
        make_identity(nc, ident_h[:])
        ones_r = const.tile([1, 128], f16, tag="onesr")
        nc.vector.memset(ones_r[:], 1.0)

        # x slices live in a ring: consumed once by the projections, then
        # the space recycles (frees 24KB/partition vs a resident [12, N])
        xs = [work.tile([128, 12, 512], f16, tag="xs", bufs=3, name=f"xs{sl}")
              for sl in range(4)]
        wqk_s = const.tile([128, 12, 2 * DK], f16, tag="wqk")
        wv_s = const.tile([128, 12, DV], f16, tag="wv")
        wrel2 = const.tile([128, 2 * DK], f16, tag="wrel2")
        pos0 = const.tile([128, SPAN], f16, tag="pos0")
        pos1 = const.tile([64, SPAN], f16, tag="pos1")
        rb2_s = const.tile([DK, 2], f32, tag="rb2")
        bo_r = const.tile([1, DIM], f16, tag="bor")
        # Arrival-matched load order: wqk + x slice 0 first (c-granular so
        # the first q/k projection starts consuming chunks as they land),
        # then pos/wrel (relkT), wv, and the remaining x slices as one
        # strided DMA each; bulky non-urgent constants (wo2, bo) last.
        if K_DMAORD == "pos_very_first":
            nc.sync.dma_start(out=wrel2[:], in_=wrel2_e[:])
            nc.sync.dma_start(out=pos0[:], in_=posT_e[0:128, :])
            nc.sync.dma_start(out=pos1[:], in_=posT_e[128:192, :])
        nc.sync.dma_start(out=wqk_s[:], in_=wqk_e[:])
        nc.sync.dma_start(out=rb2_s[:], in_=rb2_e[:])
        if K_DMAORD != "pos_very_first":
            nc.sync.dma_start(out=wrel2[:], in_=wrel2_e[:])
        if K_DMAORD == "pos_first":
            if K_POSHI:
                # high pos columns first: relk chunks sc3-7 cover the windows
                # of the G-lead tiles (cols >= 1664), so the shear pipeline
                # starts ~2us earlier; low columns follow after x slice 0
                nc.sync.dma_start(out=pos0[:, 1536:SPAN], in_=posT_e[0:128, 1536:SPAN])
                nc.sync.dma_start(out=pos1[:, 1536:SPAN], in_=posT_e[128:192, 1536:SPAN])
            else:
                nc.sync.dma_start(out=pos0[:], in_=posT_e[0:128, :])
                nc.sync.dma_start(out=pos1[:], in_=posT_e[128:192, :])
        for c in range(12):
            nc.sync.dma_start(out=xs[0][:, c, :], in_=xT_e[128 * c:128 * (c + 1), 0:512])
        if K_DMAORD == "pos_first" and K_POSHI:
            nc.sync.dma_start(out=pos0[:, 0:1536], in_=posT_e[0:128, 0:1536])
            nc.sync.dma_start(out=pos1[:, 0:1536], in_=posT_e[128:192, 0:1536])
        if K_DMAORD != "pos_first":
            nc.sync.dma_start(out=pos0[:], in_=posT_e[0:128, :])
            nc.sync.dma_start(out=pos1[:], in_=posT_e[128:192, :])
        nc.sync.dma_start(out=wv_s[:], in_=wvp_e[:])
        for sl in range(1, 4):
            nc.sync.dma_start(
                out=xs[sl][:],
                in_=bass.AP(xT_e, 512 * sl,
                            [[N, 128], [N * 128, 12], [1, 512]]))
        # wof/bo are deferred: emitted mid-loop so their ~15us of transfers
        # queue behind the first G-shear round trips on the DMA engines
        # (they are only needed from the first all-to-all stage onward)
        wof_p = [work.tile([128, 4, DIM], f16, tag="xs", bufs=3, name=f"wof{j}")
                 for j in range(3)]

        def emit_wof_load():
            nc.sync.dma_start(out=bo_r[:], in_=bo_e[:])
            for j in range(3):
                nc.sync.dma_start(out=wof_p[j][:],
                                  in_=wof_e[:, 4 * j:4 * (j + 1), :])

        # rel_k^T [64, 4095(+1 pad)] f16
        relkT = const.tile([DK, SPAN + 1], f16, tag="relkT")

        def emit_relk(scs):
            for sc in scs:
                w = min(512, SPAN - 512 * sc)
                pr = psum.tile([DK, 512], f32, tag="pg", name=f"pr{sc}")
                nc.tensor.matmul(pr[:, 0:w], wrel2[:, 0:DK],
                                 pos0[:, 512 * sc:512 * sc + w],
                                 start=True, stop=False)
                nc.tensor.matmul(pr[:, 0:w], wrel2[0:64, DK:2 * DK],
                                 pos1[:, 512 * sc:512 * sc + w],
                                 start=False, stop=True)
                nc.vector.tensor_copy(relkT[:, 512 * sc:512 * sc + w], pr[:, 0:w])


        # ---- projections ----
        qcT = const.tile([DK, N], f16, tag="qcT")  # (q*s + rcb)^T
        qpT = const.tile([DK, N], f16, tag="qpT")  # (q*s + rpb)^T
        kT = const.tile([DK, N], f16, tag="kT")
        # j-tile jt at [:, jt, :]; column DV is all-ones so the PV matmul
        # also accumulates the softmax row sums (po[:, DV])
        vb = const.tile([128, NIT, DV + 1], bf16, tag="vb")
        nc.vector.memset(vb[:, :, DV:DV + 1], 1.0)

        def emit_qk(ic):
            pq = psum.tile([128, 512], f32, tag="pc", name=f"pq{ic}")
            for c in range(12):
                nc.tensor.matmul(pq[:], wqk_s[:, c, :],
                                 xs[ic][:, c, :],
                                 start=(c == 0), stop=(c == 11))
            nc.scalar.activation(qcT[:, 512 * ic:512 * (ic + 1)], pq[0:DK, :],
                                 mybir.ActivationFunctionType.Identity,
                                 bias=rb2_s[:, 0:1], scale=1.0)
            nc.scalar.activation(qpT[:, 512 * ic:512 * (ic + 1)], pq[0:DK, :],
                                 mybir.ActivationFunctionType.Identity,
                                 bias=rb2_s[:, 1:2], scale=1.0)
            nc.vector.tensor_copy(kT[:, 512 * ic:512 * (ic + 1)], pq[DK:2 * DK, :])

        # ---- main attention loop (G stage software-pipelined K_LEAD ahead) ----
        # Exp-split: the DRAM shear round trip carries exp(R) (bf16), so the
        # shifted rel logits never need adding to the content logits on PE;
        # instead E = exp(C) * exp(R)_shifted on the otherwise-idle Pool
        # engine (with fused rowsum accumulation).
        def emit_g(it):
            """Rel-logit window matmuls + exp + DRAM shift round trip."""
            i0 = IT * it
            w0 = (N - IT) - i0  # window start s0 = 1920 - i0
            gwin = work.tile([128, GPITCH], bf16, tag="gwin", bufs=K_GB, name=f"gwin{it}")
            qorder = [0, 2, 1, 3] if K_GORD else [0, 1, 2, 3]
            for q in qorder:
                pg = psum.tile([128, JC], f32, tag="pg", name=f"pg{it}_{q}")
                nc.tensor.matmul(pg[:], qpT[:, i0:i0 + IT],
                                 relkT[:, w0 + JC * q:w0 + JC * (q + 1)],
                                 start=True, stop=True)
                nc.scalar.activation(gwin[:, JC * q:JC * (q + 1)], pg[:],
                                     mybir.ActivationFunctionType.Exp)
            pg2 = psum.tile([128, IT], f32, tag="pp", bufs=K_PPB, name=f"pg2_{it}")
            nc.tensor.matmul(pg2[:, 0:IT - 1], qpT[:, i0:i0 + IT],
                             relkT[:, w0 + 4 * JC:w0 + GW], start=True, stop=True)
            nc.scalar.activation(gwin[:, 4 * JC:GW], pg2[:, 0:IT - 1],
                                 mybir.ActivationFunctionType.Exp)
            gd = gds[it % NGD]
            nc.sync.dma_start(out=gd[:, 0:GW], in_=gwin[:, 0:GW])
            diag = bass.AP(gd, 127, [[GW, 128], [1, N]])
            rel = work.tile([128, N], bf16, tag="rel", bufs=K_RB, name=f"rel{it}")
            nc.sync.dma_start(out=rel[:], in_=diag)
            return rel

        def emit_logits(it, rel):
            """Content logits, exp, E = expC * expR_shifted -> E.

            The softmax row sums ride the PV stage: ones-column matmuls
            against the transposed E blocks accumulate into a spare PSUM
            column of the PV accumulator (ap_size 1, ~free on PE)."""
            i0 = IT * it
            E = work.tile([128, N], bf16, tag="E", bufs=K_EB, name=f"E{it}")
            E0 = work.tile([128, N], bf16, tag="E0", bufs=3, name=f"E0_{it}")
            rs4 = None
            if K_RSUM == "dve" and K_EMUL != "dvestt":
                rs4 = work.tile([128, NJC], f32, tag="rs4", bufs=2,
                                name=f"rs4_{it}")
            for jc in range(NJC):
                j0 = JC * jc
                pc = psum.tile([128, JC], f32, tag="pc", name=f"pc{it}_{jc}")
                nc.tensor.matmul(pc[:], qcT[:, i0:i0 + IT], kT[:, j0:j0 + JC],
                                 start=True, stop=True)
                nc.scalar.activation(E0[:, j0:j0 + JC], pc[:],
                                     mybir.ActivationFunctionType.Exp)
                if K_EMUL == "dvestt":
                    # fused multiply + rowsum accumulate on DVE
                    nc.vector.scalar_tensor_tensor(
                        E[:, j0:j0 + JC], E0[:, j0:j0 + JC], 1.0,
                        rel[:, j0:j0 + JC], mybir.AluOpType.mult,
                        mybir.AluOpType.mult, accum_out=rs4[:, jc:jc + 1])
                else:
                    for hh in range(K_ESPLIT):
                        HW2 = JC // K_ESPLIT
                        h0 = j0 + HW2 * hh
                        eng = nc.gpsimd if K_EMUL == "pool" else nc.vector
                        eng.tensor_tensor(E[:, h0:h0 + HW2],
                                          E0[:, h0:h0 + HW2],
                                          rel[:, h0:h0 + HW2],
                                          mybir.AluOpType.mult)
                    if K_RSUM == "dve":
                        nc.vector.reduce_sum(rs4[:, jc:jc + 1],
                                             E[:, j0:j0 + JC],
                                             axis=mybir.AxisListType.X)
            if K_RSUM == "dve":
                rs = work.tile([128, 1], f32, tag="rs", bufs=2, name=f"rs{it}")
                nc.vector.reduce_sum(rs[:], rs4[:], axis=mybir.AxisListType.X)
                rcp = work.tile([128, 1], f32, tag="rcp", bufs=2,
                                name=f"rcp{it}")
                nc.vector.reciprocal(rcp[:], rs[:])
                return (E, rcp)
            return (E,)

        def emit_pv(it, E, rcp=None):
            """PV (+ rowsum column) + ohT all-to-all send for tile it."""
            i0 = IT * it
            # PV: accumulate over j tiles with transposed E blocks.
            # 4 transposes share one PSUM tile so one copy moves 4 blocks.
            # Column DV holds the softmax row sums (ones-matmul accumulation).
            po = psum.tile([128, DV + 1], f32, tag="po", bufs=K_POB, name=f"po{it}")

            def emit_tr(jq):
                etb4 = work.tile([128, 4, 128], bf16, tag="etb4", bufs=3,
                                 name=f"etb4_{it}_{jq}")
                if jq < K_TRDMA:
                    # offload this transpose group to the DMA xbar: one
                    # [128,512] -> [128,4,128] dma transpose replaces 4 PE
                    # transposes + a DVE copy (PE sequencer is saturated)
                    nc.sync.dma_start_transpose(
                        out=etb4[:], in_=E[:, JC * jq:JC * (jq + 1)])
                    return etb4
                pt4 = psum.tile([128, 4, 128], bf16, tag="tr4", bufs=K_TRB,
                                name=f"pt4_{it}_{jq}")
                for q in range(4):
                    jt = 4 * jq + q
                    nc.tensor.transpose(pt4[:, q, :], E[:, IT * jt:IT * (jt + 1)],
                                        ident_b[:])
                on_act = (K_ETB == "act" or (K_ETB == "alt" and jq % 2 == 1))
                if on_act:
                    nc.scalar.copy(etb4[:], pt4[:])
                else:
                    nc.vector.tensor_copy(etb4[:], pt4[:])
                return etb4

            etb_q = [emit_tr(0), emit_tr(1)]
            for jq in range(NIT // 4):
                etb4 = etb_q.pop(0)
                if jq + 2 < NIT // 4:
                    etb_q.append(emit_tr(jq + 2))
                for q in range(4):
                    jt = 4 * jq + q
                    nc.tensor.matmul(po[:], etb4[:, q, :], vb[:, jt, :],
                                     start=(jt == 0), stop=(jt == NIT - 1),
                                     skip_group_check=True)
            if rcp is None:
                rcp = work.tile([128, 1], f32, tag="rcp", bufs=2,
                                name=f"rcp{it}")
                nc.vector.reciprocal(rcp[:], po[:, DV:DV + 1])
            oh = work.tile([128, DV], f16, tag="oh", name=f"oh{it}")
            nc.vector.tensor_scalar(oh[:], po[:, 0:DV], rcp[:], None,
                                    mybir.AluOpType.mult)
            # transpose oh -> ohT (c-chunks of 96)
            ohT = work.tile([96, 2, 128], f16, tag="ohT", name=f"ohT{it}")
            for h in range(2):
                pth = psum.tile([96, 128], f16, tag="tr4", bufs=K_TRB,
                                name=f"pth{it}_{h}")
                nc.tensor.transpose(pth[:], oh[:, 96 * h:96 * (h + 1)], ident_h[:])
                if K_OHT == "act":
                    nc.scalar.copy(ohT[:, h, :], pth[:])
                else:
                    nc.vector.tensor_copy(ohT[:, h, :], pth[:])
            # send this tile's ohT into its all-to-all staging slot
            ag = it // 8
            nc.sync.dma_start(
                out=bass.AP(a2a_ins[ag], (it % 8) * DV * IT,
                            [[IT, 96], [96 * IT, 2], [1, IT]]),
                in_=ohT[:])

        def emit_logits_chunk(it, rel, E, E0, jc):
            i0 = IT * it
            j0 = JC * jc
            pc = psum.tile([128, JC], f32, tag="pc", name=f"pc{it}_{jc}")
            nc.tensor.matmul(pc[:], qcT[:, i0:i0 + IT], kT[:, j0:j0 + JC],
                             start=True, stop=True)
            nc.scalar.activation(E0[:, j0:j0 + JC], pc[:],
                                 mybir.ActivationFunctionType.Exp)
            nc.vector.tensor_tensor(E[:, j0:j0 + JC], E0[:, j0:j0 + JC],
                                    rel[:, j0:j0 + JC], mybir.AluOpType.mult)

        def emit_pv_interleaved(itp, Ep, it, rel):
            """emit_pv(itp) with logits chunks of tile it woven between the
            transpose/PV groups, so ACT/DVE start tile it while PE runs
            tile itp."""
            E = work.tile([128, N], bf16, tag="E", bufs=K_EB, name=f"E{it}")
            E0 = work.tile([128, N], bf16, tag="E0", bufs=3, name=f"E0_{it}")
            po = psum.tile([128, DV + 1], f32, tag="po", bufs=K_POB,
                           name=f"po{itp}")

            def emit_tr(jq):
                pt4 = psum.tile([128, 4, 128], bf16, tag="tr4", bufs=K_TRB,
                                name=f"pt4_{itp}_{jq}")
                for q in range(4):
                    jt = 4 * jq + q
                    nc.tensor.transpose(pt4[:, q, :],
                                        Ep[:, IT * jt:IT * (jt + 1)],
                                        ident_b[:])
                etb4 = work.tile([128, 4, 128], bf16, tag="etb4", bufs=3,
                                 name=f"etb4_{itp}_{jq}")
                nc.vector.tensor_copy(etb4[:], pt4[:])
                return etb4

            emit_logits_chunk(it, rel, E, E0, 0)
            etb_q = [emit_tr(0), emit_tr(1)]
            for jq in range(NIT // 4):
                if jq < NJC - 1:
                    emit_logits_chunk(it, rel, E, E0, jq + 1)
                etb4 = etb_q.pop(0)
                if jq + 2 < NIT // 4:
                    etb_q.append(emit_tr(jq + 2))
                for q in range(4):
                    jt = 4 * jq + q
                    nc.tensor.matmul(po[:], etb4[:, q, :], vb[:, jt, :],
                                     start=(jt == 0), stop=(jt == NIT - 1),
                                     skip_group_check=True)
            rcp = work.tile([128, 1], f32, tag="rcp", bufs=2, name=f"rcp{itp}")
            nc.vector.reciprocal(rcp[:], po[:, DV:DV + 1])
            oh = work.tile([128, DV], f16, tag="oh", name=f"oh{itp}")
            nc.vector.tensor_scalar(oh[:], po[:, 0:DV], rcp[:], None,
                                    mybir.AluOpType.mult)
            ohT = work.tile([96, 2, 128], f16, tag="ohT", name=f"ohT{itp}")
            for h in range(2):
                pth = psum.tile([96, 128], f16, tag="tr4", bufs=K_TRB,
                                name=f"pth{itp}_{h}")
                nc.tensor.transpose(pth[:], oh[:, 96 * h:96 * (h + 1)],
                                    ident_h[:])
                nc.vector.tensor_copy(ohT[:, h, :], pth[:])
            ag = itp // 8
            nc.sync.dma_start(
                out=bass.AP(a2a_ins[ag], (itp % 8) * DV * IT,
                            [[IT, 96], [96 * IT, 2], [1, IT]]),
                in_=ohT[:])
            return (E,)

        def emit_a2a_comm(ag):
            # Exchange ohT tiles: slot o -> core o (owner of tile 8*ag+o),
            # then gather the 8 heads' [192, 128] blocks into the
            # [128, 12, 128] dv-swizzled stationary layout: chunk h (0-7) =
            # head h dv 0:128; chunk 8+k = heads (2k, 2k+1) dv 128:192.
            if collective:
                nc.gpsimd.collective_compute(
                    "AllToAll",
                    mybir.AluOpType.bypass,
                    replica_groups=[list(range(num_cores))],
                    ins=[a2a_ins[ag][:]],
                    outs=[a2a_outs[ag][:]],
                )
                cc_src = a2a_outs[ag]
            else:
                cc_src = a2a_ins[ag]  # timing mirror: same local read traffic
            agb = work.tile([128, 12, IT], f16, tag="agb", bufs=2, name=f"agb{ag}")
            HB = DV * IT  # one head slot
            nc.sync.dma_start(
                out=agb[:, 0:8, :],
                in_=bass.AP(cc_src, 0, [[IT, 128], [HB, 8], [1, IT]]))
            for b in range(2):
                nc.sync.dma_start(
                    out=agb[64 * b:64 * (b + 1), 8:12, :],
                    in_=bass.AP(cc_src, b * HB + 128 * IT,
                                [[IT, 64], [2 * HB, 4], [1, IT]]))
            return agb

        def emit_outproj(ag, agb, tag, bufs):
            # Final out rows for owned tile 8*ag + core_id: agb @ woP + bo.
            fin = work.tile([128, 3, JC], f32, tag="fin", bufs=K_FB, name=f"fin{ag}")
            rings = [(tag, bufs), ("po", K_POB), (tag, bufs)] if K_OPALT \
                else [(tag, bufs)] * 3
            for cc in range(3):
                rt, rb = rings[cc]
                pp = psum.tile([128, JC], f32, tag=rt, bufs=rb,
                               name=f"ppo{ag}_{cc}")
                # bias start-pass: ones-column x bo row seeds PSUM with the
                # broadcast bias, so evacuation is a plain copy on the
                # (drain-idle) ACT engine instead of a DVE add
                nc.tensor.matmul(pp[:], ones_r[:, 0:128],
                                 bo_r[:, JC * cc:JC * (cc + 1)],
                                 start=True, stop=False)
                for kc in range(12):
                    nc.tensor.matmul(pp[:], agb[:, kc, :],
                                     wof_p[kc // 4][:, kc % 4, JC * cc:JC * (cc + 1)],
                                     start=False, stop=(kc == 11))
                if K_FINE == "act":
                    nc.scalar.copy(fin[:, cc, :], pp[:])
                else:
                    nc.vector.tensor_copy(fin[:, cc, :], pp[:])
                nc.sync.dma_start(out=out_e[IT * ag:IT * (ag + 1),
                                            JC * cc:JC * (cc + 1)],
                                  in_=fin[:, cc, :])

        # drive: G leads by two tiles, PV trails exp by one tile
        def emit_v(jt):
            pv = psum.tile([128, DV], f32, tag="po", bufs=K_POB, name=f"pv{jt}")
            for c in range(12):
                nc.tensor.matmul(pv[:], xs[jt // 4][:, c, IT * (jt % 4):IT * (jt % 4 + 1)],
                                 wv_s[:, c, :], start=(c == 0), stop=(c == 11))
            nc.vector.tensor_copy(vb[:, jt, 0:DV], pv[:])

        # Startup, arrival-matched: qk(ic0) as soon as x slice 0 lands, the
        # G lead tiles next (they only need qpT cols 0:128 + relkT), then
        # V tiles / remaining qk chunks in x-slice arrival order.
        # p-state warmup: the PE ramps to full speed only after ~3us of
        # continuous execution; idle until the pos DMAs land would leave the
        # first (DMA-window-critical) projections running 1.5-2x slow.
        # Identity self-matmuls need no external data and keep the PE hot.
        for wu in range(K_WARM):
            pw = psum.tile([128, 128], f32, tag="tr4", bufs=K_TRB,
                           name=f"warm{wu}")
            nc.tensor.matmul(pw[:], ident_b[:], ident_b[:],
                             start=True, stop=True)
        if K_EMIT0 == "qk_first":
            emit_qk(0)
            emit_relk(range(8))
        elif K_POSHI:
            emit_relk(range(3, 8))
            emit_qk(0)
        else:
            emit_relk(range(4))
            emit_relk(range(4, 8))
            emit_qk(0)
        if K_GILV:
            # startup G-lead tiles interleaved with the V/QK blocks so the
            # in-order PE queue never parks behind a not-yet-ready stage
            rel_q = [emit_g(0), emit_g(1)]
            for jt in range(4):
                emit_v(jt)
            rel_q.append(emit_g(2))
            for ic in range(1, 4):
                emit_qk(ic)
                if ic == 1 and K_LEAD > 3:
                    rel_q.append(emit_g(3))
                for jt in range(4 * ic, 4 * ic + 4):
                    emit_v(jt)
        else:
            rel_q = [emit_g(i) for i in range(K_LEAD)]
            for jt in range(4):
                emit_v(jt)
            for ic in range(1, 4):
                emit_qk(ic)
                for jt in range(4 * ic, 4 * ic + 4):
                    emit_v(jt)

        pv_q = []
        for it in range(NIT):
            rel = rel_q.pop(0)
            if K_GPOS == "first" and it + K_LEAD < NIT:
                rel_q.append(emit_g(it + K_LEAD))
            pv_now = K_ORDER == "pv_first" and (it < NIT - 1 or not K_DRAIN)
            if K_ILV and pv_now and len(pv_q) >= K_TRAIL:
                # fine-grained interleave: logits chunks of tile it between
                # the transpose/PV groups of tile it-1
                itp, eo = pv_q.pop(0)
                if it == K_WOFIT:
                    emit_wof_load()
                if it == 9:
                    agb0 = emit_a2a_comm(0)
                pv_q.append((it, emit_pv_interleaved(itp, eo[0], it, rel)))
                continue
            if pv_now and len(pv_q) >= K_TRAIL:
                itp, eo = pv_q.pop(0)
                emit_pv(itp, *eo)
            if it == K_WOFIT:
                emit_wof_load()
            if it == K_COMM0:
                agb0 = emit_a2a_comm(0)
            pv_q.append((it, emit_logits(it, rel)))
            if it == 10 and K_OP0 == "steady":
                emit_outproj(0, agb0, K_OP0RING, K_PCB if K_OP0RING == "pc" else K_PPB)
            if not pv_now and len(pv_q) > K_TRAIL:
                itp, eo = pv_q.pop(0)
                emit_pv(itp, *eo)
            if K_GPOS == "late" and it + K_LEAD < NIT:
                rel_q.append(emit_g(it + K_LEAD))
        for itp, eo in pv_q:
            emit_pv(itp, *eo)
        # group-0 outproj in the drain: its matmuls overlap the group-1
        # exchange -> gather DMA chain and keep the PE p-state warm
        agb1 = emit_a2a_comm(1)
        if K_OP0 == "drain":
            emit_outproj(0, agb0, K_OP0RING, K_PCB if K_OP0RING == "pc" else K_PPB)
        emit_outproj(1, agb1, "pg", 2)

    nc.compile()
    return nc


_CACHE: dict = {}


def _get_nc():
    if "nc" not in _CACHE:
        _CACHE["nc"] = build_nc()
    return _CACHE["nc"]


def _shard_inputs(x, Wq, Wk, Wv, Wrel, rel_content_bias, rel_pos_bias, Wo, bo):
    posT = np.ascontiguousarray(_positions().T).astype(np.float16)  # [192, 4095]
    xT = np.ascontiguousarray(
        np.asarray(x, np.float32).reshape(N, DIM).T).astype(np.float16)
    # dv-swizzled full Wo: chunk h (0-7) = head h dv 0:128; chunk 8+k =
    # heads (2k, 2k+1) dv 128:192 stacked 64+64 (matches emit_a2a_comm).
    woP = np.empty((DIM, DIM), np.float32)
    for h in range(8):
        woP[128 * h:128 * (h + 1)] = Wo[DV * h:DV * h + 128]
    for k in range(4):
        woP[1024 + 128 * k:1024 + 128 * k + 64] = Wo[DV * 2 * k + 128:DV * 2 * k + DV]
        woP[1024 + 128 * k + 64:1024 + 128 * (k + 1)] = \
            Wo[DV * (2 * k + 1) + 128:DV * (2 * k + 1) + DV]
    wof = np.ascontiguousarray(
        woP.reshape(12, 128, DIM).transpose(1, 0, 2)).astype(np.float16)
    bo_row = np.asarray(bo, np.float16).reshape(1, DIM)
    in_maps = []
    for h in range(NCORES):
        wq = (Wq[:, DK * h:DK * (h + 1)] * SCALE).astype(np.float16)
        wk = Wk[:, DK * h:DK * (h + 1)].astype(np.float16)
        wqk = np.concatenate(
            [wq.reshape(12, 128, DK), wk.reshape(12, 128, DK)], axis=2)
        wvp = Wv[:, DV * h:DV * (h + 1)].astype(np.float16).reshape(12, 128, DV)
        wrel = Wrel[:, DK * h:DK * (h + 1)].astype(np.float16)
        wrel2 = np.zeros((128, 2 * DK), np.float16)
        wrel2[:, 0:DK] = wrel[0:128]
        wrel2[0:64, DK:2 * DK] = wrel[128:192]
        rb2 = np.stack([rel_content_bias[0, h, 0, :],
                        rel_pos_bias[0, h, 0, :]], axis=1).astype(np.float32)
        in_maps.append({
            "xT": xT,
            "wqk": np.ascontiguousarray(wqk.transpose(1, 0, 2)),
            "wvp": np.ascontiguousarray(wvp.transpose(1, 0, 2)),
            "wrel2": wrel2,
            "posT": posT,
            "rb2": np.ascontiguousarray(rb2),
            "wof": wof,
            "bo": bo_row,
        })
    return in_maps


def kernel(**inputs) -> np.ndarray:
    from concourse.bass_utils import run_bass_kernel_spmd

    inputs = {k: np.asarray(v) for k, v in inputs.items()}
    nc = _get_nc()
    in_maps = _shard_inputs(**inputs)
    res = run_bass_kernel_spmd(nc, in_maps, list(range(NCORES)))
    # core c owns row-tiles {c, 8+c}: rows [128c, 128c+128) and
    # [1024+128c, 1024+128c+128)
    out = np.empty((N, DIM), np.float32)
    for c in range(NCORES):
        oc = np.asarray(res.results[c]["out"])
        out[IT * c:IT * (c + 1), :] = oc[0:IT, :]
        out[1024 + IT * c:1024 + IT * (c + 1), :] = oc[IT:2 * IT, :]
    return out.reshape(1, N, DIM)

